# revision 49
# baseline (speedup 1.0000x reference)
import math
import os
from contextlib import ExitStack

import numpy as np

N, T, D, H = 512, 128, 512, 512
T = int(os.environ.get("KERNEL_T", T)) if "KERNEL_T" in os.environ else T
NC = 8
n = N // NC          # 64 samples per core
H4 = 4 * H           # 2048
P16 = 16             # attention locations
SCALE = 1.0 / math.sqrt(H)
TRACE = os.environ.get("KERNEL_TRACE", "0") == "1"
NOBCAST = os.environ.get("KERNEL_NOBCAST", "0") == "1"
STUBATTN = os.environ.get("KERNEL_STUBATTN", "0") == "1"
SIMPLECELL = os.environ.get("KERNEL_SIMPLECELL", "0") == "1"
LAST_EXEC_NS = None

_cache = {}


def _build_kernel():
    if "nc" in _cache:
        return _cache["nc"]

    import concourse.bass as bass
    import concourse.tile as tile
    from concourse import bacc, mybir

    f32 = mybir.dt.float32
    bf16 = mybir.dt.bfloat16
    ALU = mybir.AluOpType
    ACTF = mybir.ActivationFunctionType
    AX = mybir.AxisListType

    nc = bacc.Bacc(
        "TRN2",
        target_bir_lowering=False,
        debug=False,
        enable_asserts=False,
        num_devices=NC,
    )

    # ---- DRAM I/O ---------------------------------------------------------
    xT = nc.dram_tensor("xT", (D, n * T), bf16, kind="ExternalInput").ap()
    # A^T chunks in (p, s) free order for the dot product, and (s, p)
    # order for the weighted-sum path (keeps innermost stride 1 on both).
    ATps = nc.dram_tensor("ATps", (128, 4 * n * P16), bf16, kind="ExternalInput").ap()
    ATsp = nc.dram_tensor("ATsp", (128, 4 * n * P16), bf16, kind="ExternalInput").ap()
    Wc = nc.dram_tensor("Wc", (128, 12 * H4), bf16, kind="ExternalInput").ap()
    bvec = nc.dram_tensor("bvec", (1, H4), bf16, kind="ExternalInput").ap()
    idb = nc.dram_tensor("idb", (n, n), bf16, kind="ExternalInput").ap()
    onesv = nc.dram_tensor("onesv", (1, 128), bf16, kind="ExternalInput").ap()
    onesK = nc.dram_tensor("onesK", (128, 1), bf16, kind="ExternalInput").ap()
    h0T_in = nc.dram_tensor("h0T", (128, 4 * n), bf16, kind="ExternalInput").ap()
    c0_in = nc.dram_tensor("c0", (n, H), bf16, kind="ExternalInput").ap()
    hs = nc.dram_tensor("hs", (T, n, H), bf16, kind="ExternalOutput").ap()

    with tile.TileContext(nc) as tc, ExitStack() as ctx:
        const_pool = ctx.enter_context(tc.tile_pool(name="const", bufs=1))
        xts_pool = ctx.enter_context(tc.tile_pool(name="xts", bufs=3))
        work = ctx.enter_context(tc.tile_pool(name="work", bufs=1))
        hbf_pool = ctx.enter_context(tc.tile_pool(name="hbf", bufs=2))
        psum_hb = ctx.enter_context(tc.tile_pool(name="psum_hb", bufs=1, space="PSUM"))
        psum_sm = ctx.enter_context(tc.tile_pool(name="psum_sm", bufs=1, space="PSUM"))
        psum_tp = ctx.enter_context(tc.tile_pool(name="psum_tp", bufs=1, space="PSUM"))

        # ---- persistent tiles --------------------------------------------
        W_sb = const_pool.tile([128, 12 * H4], bf16)
        nc.sync.dma_start(W_sb[:], Wc[:])
        b_sb = const_pool.tile([1, H4], bf16)
        nc.sync.dma_start(b_sb[:], bvec[:])
        id_sb = const_pool.tile([n, n], bf16)
        nc.sync.dma_start(id_sb[:], idb[:])
        A_ps = const_pool.tile([128, 4 * n * P16], bf16)   # [q, hc, p, s]
        nc.sync.dma_start(A_ps[:], ATps[:])
        A_sp = const_pool.tile([128, 4 * n * P16], bf16)   # [q, hc, s, p]
        nc.sync.dma_start(A_sp[:], ATsp[:])
        ones_r = const_pool.tile([1, 128], bf16)
        nc.sync.dma_start(ones_r[:], onesv[:])
        ones_k = const_pool.tile([128, 1], bf16)
        nc.sync.dma_start(ones_k[:], onesK[:])

        hT = const_pool.tile([128, 4 * n], bf16)           # [q, hc, s]
        nc.sync.dma_start(hT[:], h0T_in[:])
        c_st = const_pool.tile([n, H], bf16)
        nc.sync.dma_start(c_st[:], c0_in[:])

        Aps_v = A_ps[:].rearrange("q (c p s) -> q c p s", c=4, p=P16)
        Asp_v = A_sp[:].rearrange("q (c s p) -> q c s p", c=4, p=P16)
        hT_v = hT[:].rearrange("q (c s) -> q c s", c=4)
        xT_r = xT.rearrange("(c q) (nn tt) -> tt c q nn", q=128, tt=T)

        # scratch tiles reused every step
        prod = work.tile([128, 4 * n * P16], bf16)         # (p, s) order
        prod_v = prod[:].rearrange("q (c p s) -> q c p s", c=4, p=P16)
        wexp = work.tile([1, n * P16], bf16)               # (s, p) order
        w_bf = work.tile([128, n * P16], bf16)             # replicated wexp
        sc1 = work.tile([1, 8 * n], bf16)                  # ssum tree scratch
        ssum = work.tile([1, n], bf16)
        rinv = work.tile([1, n], bf16)
        rinv_rep = work.tile([128, n], bf16)
        prodA = work.tile([128, 4 * n * P16], bf16)        # (s, p) order
        prodA_v = prodA[:].rearrange("q (c s p) -> q c s p", c=4, p=P16)
        attnT_bf = work.tile([128, 4 * n], bf16)
        tau = work.tile([n, 3 * H], bf16)
        uu_o = work.tile([n, H], bf16)                     # 0.5*tau_o + 0.5
        gg = work.tile([n, H], bf16)
        m1 = work.tile([n, H], bf16)
        m2 = work.tile([n, H], bf16)
        tc_t = work.tile([n, H], bf16)
        acc_d = work.tile([n, 1], f32)
        u_i = work.tile([n, H], bf16)
        u_f = work.tile([n, H], bf16)

        W_k = lambda c, j: W_sb[:, H4 * c + 512 * j:H4 * c + 512 * (j + 1)]

        def emit_xb_mms(hb, xts, js):
            # x@Wx + b: first writes of the accumulation group (start=True
            # on the initial write of each column block)
            for j in js:
                cols = slice(512 * j, 512 * (j + 1))
                for ci, c in enumerate([8, 9, 10, 11, 12]):
                    if c < 12:
                        lhsT = xts[:, n * (c - 8):n * (c - 7)]
                        rhs = W_k(c, j)
                    else:
                        lhsT = ones_r[:, 0:n]
                        rhs = b_sb[:, cols]
                    nc.tensor.matmul(hb[:, cols], lhsT, rhs,
                                     start=(ci == 0), stop=False)

        # prologue: x/b matmuls for t=0
        xts_cur = xts_pool.tile([128, 4 * n], bf16)
        for ci in range(4):
            nc.sync.dma_start(xts_cur[:, n * ci:n * (ci + 1)], xT_r[0, ci])
        hb = psum_hb.tile([n, H4], f32)
        emit_xb_mms(hb, xts_cur, (0, 1, 2, 3))

        tp_prev = None
        for t in range(T):
            # prefetch x_{t+1}^T
            if t + 1 < T:
                xts_nxt = xts_pool.tile([128, 4 * n], bf16)
                for ci in range(4):
                    nc.sync.dma_start(xts_nxt[:, n * ci:n * (ci + 1)],
                                      xT_r[t + 1, ci])

            # ---- dot[p,s] = sum_h A[h,p,s] hT[h,s]; PE ones-reduce --------
            if NOBCAST:
                wrep2 = psum_sm.tile([128, n * P16], f32)
                dotp = wrep2[0:1, :]
            else:
                dotp = psum_sm.tile([1, n * P16], f32)
            half = n * P16 // 2
            # read h^T straight from the transpose PSUM (skips waiting on
            # the SBUF copy); first step reads the DMA'd h0T instead
            hsrc = hT_v if tp_prev is None else \
                tp_prev[:].rearrange("q (c s) -> q c s", c=4)
            if not STUBATTN:
                for ci in range(4):
                    h_b = hsrc[:, ci].rearrange("q (r s) -> q r s", r=1) \
                                     .broadcast_to([128, P16, n])
                    nc.vector.tensor_tensor(prod_v[:, ci], Aps_v[:, ci], h_b,
                                            ALU.mult)
                    for hi in range(2):
                        nc.tensor.matmul(
                            dotp[:, half * hi:half * (hi + 1)],
                            ones_k[:],
                            prod[:, n * P16 * ci + half * hi:
                                    n * P16 * ci + half * (hi + 1)],
                            start=(ci == 0),
                            stop=(ci == 3),
                        )

            # second half of next-psum x/b matmuls (PE filler behind the
            # ones-reduce; their psum group was opened last iteration)
            if t > 0:
                emit_xb_mms(hb, xts_cur, (2, 3))

            # h@Wh: fills PE while the softmax chain runs
            for j in range(4):
                cols = slice(512 * j, 512 * (j + 1))
                for c in range(4):
                    nc.tensor.matmul(hb[:, cols], hT[:, n * c:n * (c + 1)],
                                     W_k(c, j), start=False, stop=False)

            # ---- softmax numerator, stored (s, p) via strided write -------
            # two s-halves so the partition broadcast pipelines behind exp
            dview = dotp.rearrange("q (p s) -> q s p", p=P16)
            if STUBATTN:
                pass
            elif NOBCAST:
                nc.scalar.activation(
                    wexp[:].rearrange("q (s p) -> q s p", p=P16),
                    dview, ACTF.Exp, scale=SCALE)
                for hi in range(2):
                    sl = slice(half * hi, half * (hi + 1))
                    nc.tensor.matmul(wrep2[:, sl], ones_r[:], wexp[:, sl],
                                     start=True, stop=True)
                    nc.scalar.activation(w_bf[:, sl], wrep2[:, sl], ACTF.Copy)
            else:
                for hi in range(2):
                    sl = slice(half * hi, half * (hi + 1))
                    ssl = slice(32 * hi, 32 * (hi + 1))
                    nc.scalar.activation(
                        wexp[:, sl].rearrange("q (s p) -> q s p", p=P16),
                        dview[:, ssl],
                        ACTF.Exp, scale=SCALE)
                    nc.gpsimd.partition_broadcast(w_bf[:, sl], wexp[:, sl])

            # denominator: single-lane tree on wexp, then replicate rinv
            we_v = wexp[:].rearrange("q (s p) -> q s p", p=P16)
            sc_v = sc1[:].rearrange("q (s p) -> q s p", p=8)
            if not STUBATTN:
                with nc.allow_low_precision(reason="softmax sums in bf16"):
                    nc.vector.tensor_tensor(sc_v[:, :, 0:8], we_v[:, :, 0:8],
                                            we_v[:, :, 8:16], ALU.add)
                    nc.vector.tensor_tensor(sc_v[:, :, 0:4], sc_v[:, :, 0:4],
                                            sc_v[:, :, 4:8], ALU.add)
                    nc.vector.tensor_tensor(sc_v[:, :, 0:2], sc_v[:, :, 0:2],
                                            sc_v[:, :, 2:4], ALU.add)
                    nc.vector.tensor_tensor(
                        ssum[:].rearrange("q (s r) -> q s r", r=1),
                        sc_v[:, :, 0:1], sc_v[:, :, 1:2], ALU.add)
                    nc.vector.reciprocal(rinv[:], ssum[:])
            if STUBATTN:
                pass
            elif NOBCAST:
                rrep = psum_tp.tile([128, n], f32)
                nc.tensor.matmul(rrep[:], ones_r[:], rinv[:],
                                 start=True, stop=True)
                nc.scalar.activation(rinv_rep[:], rrep[:], ACTF.Copy)
            else:
                nc.gpsimd.partition_broadcast(rinv_rep[:], rinv[:])

            # ---- attn chunks: TT + in-place tree reduce + normalize -------
            # followed immediately by that chunk's 4 matmuls (c-outer);
            # chunks 1 and 3 multiply on Pool so DVE isn't the serial
            # bottleneck of the chunk phase
            w_v = w_bf[:].rearrange("q (s p) -> q s p", p=P16)
            with nc.allow_low_precision(reason="16-term attn sum in bf16"):
                if STUBATTN:
                    nc.scalar.activation(attnT_bf[:], hT[:], ACTF.Copy)
                for ci in range(4):
                    if STUBATTN:
                        pass
                    else:
                        pv = prodA_v[:, ci]
                        nc.vector.tensor_tensor(pv, Asp_v[:, ci], w_v,
                                                ALU.mult)
                        nc.vector.tensor_tensor(pv[:, :, 0:8], pv[:, :, 0:8],
                                                pv[:, :, 8:16], ALU.add)
                        nc.vector.tensor_tensor(pv[:, :, 0:4], pv[:, :, 0:4],
                                                pv[:, :, 4:8], ALU.add)
                        nc.vector.tensor_tensor(pv[:, :, 0:2], pv[:, :, 0:2],
                                                pv[:, :, 2:4], ALU.add)
                        nc.vector.tensor_tensor(pv[:, :, 0:1], pv[:, :, 0:1],
                                                pv[:, :, 1:2], ALU.add)
                        nc.vector.tensor_tensor(
                            attnT_bf[:, n * ci:n * (ci + 1)],
                            pv[:, :, 0],
                            rinv_rep[:], ALU.mult)
                    # this chunk's contribution to all column blocks; the
                    # last chunk's four matmuls run back-to-back BEFORE any
                    # gate reads hb (a gate read between them would force a
                    # tile-level WAR stall on the psum tile)
                    for j in (0, 1, 3, 2):
                        cols = slice(512 * j, 512 * (j + 1))
                        nc.tensor.matmul(
                            hb[:, cols],
                            attnT_bf[:, n * ci:n * (ci + 1)],
                            W_k(4 + ci, j),
                            start=False, stop=(ci == 3))

            # ---- gates; sigma(z) = 0.5*tanh(z/2) + 0.5 --------------------
            nc.scalar.activation(tau[:, 0:2 * H], hb[:, 0:2 * H],
                                 ACTF.Tanh, scale=0.5)
            nc.scalar.activation(gg[:], hb[:, 3 * H:4 * H], ACTF.Tanh)
            nc.scalar.activation(tau[:, 2 * H:3 * H], hb[:, 2 * H:3 * H],
                                 ACTF.Tanh, scale=0.5)
            nc.vector.tensor_scalar(uu_o[:], tau[:, 2 * H:3 * H],
                                    0.5, 0.5, ALU.mult, ALU.add)

            # ---- cell: c' = sig(f)c + sig(i)g ; h = sig(o)*tanh(c') -------
            # (plain TS/TT ops only: scalar_tensor_tensor and
            # tensor_tensor_reduce fault on hardware here)
            with nc.allow_low_precision(reason="cell state in bf16"):
                nc.vector.tensor_scalar(u_i[:], tau[:, 0:H],
                                        0.5, 0.5, ALU.mult, ALU.add)
                nc.vector.tensor_scalar(u_f[:], tau[:, H:2 * H],
                                        0.5, 0.5, ALU.mult, ALU.add)
                nc.vector.tensor_tensor(m2[:], u_i[:], gg[:], ALU.mult)
                nc.vector.tensor_tensor(m1[:], u_f[:], c_st[:], ALU.mult)
                nc.vector.tensor_tensor(c_st[:], m1[:], m2[:], ALU.add)
            nc.scalar.activation(tc_t[:], c_st[:], ACTF.Tanh)
            h_bf = hbf_pool.tile([n, H], bf16)
            nc.vector.tensor_tensor(h_bf[:], uu_o[:], tc_t[:], ALU.mult)

            nc.sync.dma_start(hs[t], h_bf[:])

            # transpose h for next step, then prime next step's x/b matmuls
            if t < T - 1:
                tp = psum_tp.tile([128, 4 * n], bf16)
                for ci in range(4):
                    nc.tensor.transpose(
                        tp[:, n * ci:n * (ci + 1)],
                        h_bf[:, 128 * ci:128 * (ci + 1)], id_sb[:])
                nc.scalar.activation(hT[:], tp[:], ACTF.Copy)
                tp_prev = tp
                hb = psum_hb.tile([n, H4], f32)
                emit_xb_mms(hb, xts_nxt, (0, 1))
                xts_cur = xts_nxt

    nc.compile()
    _cache["nc"] = nc
    return nc


def kernel(x, A, Wx, Wh, Wattn, b):
    import ml_dtypes
    from concourse import bass_utils

    nc = _build_kernel()
    bft = ml_dtypes.bfloat16

    Wcat = np.concatenate([np.asarray(Wh), np.asarray(Wattn), np.asarray(Wx)],
                          axis=0)                         # (1536, 2048)
    Wc_host = np.ascontiguousarray(
        Wcat.reshape(12, 128, H4).transpose(1, 0, 2).reshape(128, 12 * H4)
    ).astype(bft)
    b_host = np.asarray(b, dtype=np.float32).reshape(1, H4).astype(bft)
    id_host = np.eye(n, dtype=np.float32).astype(bft)
    ones_host = np.ones((1, 128), dtype=np.float32).astype(bft)
    onesK_host = np.ones((128, 1), dtype=np.float32).astype(bft)

    A_np = np.asarray(A, dtype=np.float32)                # (N, H, 4, 4)
    x_np = np.asarray(x, dtype=np.float32)[:, :T]

    in_maps = []
    for k in range(NC):
        xc = x_np[n * k:n * (k + 1)]                      # (64, T, D)
        Ac = A_np[n * k:n * (k + 1)].reshape(n, H, P16)   # (64, 512, 16)
        xT_host = np.ascontiguousarray(
            xc.transpose(2, 0, 1).reshape(D, n * T)).astype(bft)
        A_hps = Ac.transpose(1, 2, 0)                     # (H, p, s)
        ATps_host = np.ascontiguousarray(
            A_hps.reshape(4, 128, P16 * n)
                 .transpose(1, 0, 2).reshape(128, 4 * n * P16)).astype(bft)
        A_hsp = Ac.transpose(1, 0, 2)                     # (H, s, p)
        ATsp_host = np.ascontiguousarray(
            A_hsp.reshape(4, 128, n * P16)
                 .transpose(1, 0, 2).reshape(128, 4 * n * P16)).astype(bft)
        h0 = Ac.mean(axis=2)                              # (64, 512)
        h0T_host = np.ascontiguousarray(
            h0.T.reshape(4, 128, n).transpose(1, 0, 2).reshape(128, 4 * n)
        ).astype(bft)
        in_maps.append({
            "xT": xT_host,
            "ATps": ATps_host,
            "ATsp": ATsp_host,
            "Wc": Wc_host,
            "bvec": b_host,
            "idb": id_host,
            "onesv": ones_host,
            "onesK": onesK_host,
            "h0T": h0T_host,
            "c0": np.ascontiguousarray(h0).astype(bft),
        })

    global LAST_EXEC_NS
    res = bass_utils.run_bass_kernel_spmd(
        nc, in_maps, core_ids=list(range(NC)), trace=TRACE)
    LAST_EXEC_NS = res.exec_time_ns

    out = np.empty((N, T, H), dtype=np.float32)
    for k in range(NC):
        hs_k = np.asarray(res.results[k]["hs"]).astype(np.float32)  # (T, n, H)
        out[n * k:n * (k + 1)] = hs_k.transpose(1, 0, 2)
    return out


# revision 59
# speedup vs baseline: 1.0105x; 1.0105x over previous
import math
import os
from contextlib import ExitStack

import numpy as np

N, T, D, H = 512, 128, 512, 512
NC = 8
n = N // NC          # 64 samples per core
H4 = 4 * H           # 2048
P16 = 16             # attention locations
SCALE = 1.0 / math.sqrt(H)
TRACE = os.environ.get("KERNEL_TRACE", "0") == "1"
NOBCAST = False
STUBATTN = False
SIMPLECELL = True
LAST_EXEC_NS = None

_cache = {}


def _build_kernel():
    if "nc" in _cache:
        return _cache["nc"]

    import concourse.bass as bass
    import concourse.tile as tile
    from concourse import bacc, mybir

    f32 = mybir.dt.float32
    bf16 = mybir.dt.bfloat16
    ALU = mybir.AluOpType
    ACTF = mybir.ActivationFunctionType
    AX = mybir.AxisListType

    nc = bacc.Bacc(
        "TRN2",
        target_bir_lowering=False,
        debug=False,
        enable_asserts=False,
        num_devices=NC,
    )

    # ---- DRAM I/O ---------------------------------------------------------
    xT = nc.dram_tensor("xT", (D, n * T), bf16, kind="ExternalInput").ap()
    # A^T chunks in (p, s) free order for the dot product, and (s, p)
    # order for the weighted-sum path (keeps innermost stride 1 on both).
    ATps = nc.dram_tensor("ATps", (128, 4 * n * P16), bf16, kind="ExternalInput").ap()
    ATsp = nc.dram_tensor("ATsp", (128, 4 * n * P16), bf16, kind="ExternalInput").ap()
    Wc = nc.dram_tensor("Wc", (128, 12 * H4), bf16, kind="ExternalInput").ap()
    bvec = nc.dram_tensor("bvec", (1, H4), bf16, kind="ExternalInput").ap()
    idb = nc.dram_tensor("idb", (n, n), bf16, kind="ExternalInput").ap()
    onesv = nc.dram_tensor("onesv", (1, 128), bf16, kind="ExternalInput").ap()
    onesK = nc.dram_tensor("onesK", (128, 1), bf16, kind="ExternalInput").ap()
    h0T_in = nc.dram_tensor("h0T", (128, 4 * n), bf16, kind="ExternalInput").ap()
    c0_in = nc.dram_tensor("c0", (n, H), bf16, kind="ExternalInput").ap()
    hs = nc.dram_tensor("hs", (T, n, H), bf16, kind="ExternalOutput").ap()

    with tile.TileContext(nc) as tc, ExitStack() as ctx:
        const_pool = ctx.enter_context(tc.tile_pool(name="const", bufs=1))
        xts_pool = ctx.enter_context(tc.tile_pool(name="xts", bufs=3))
        work = ctx.enter_context(tc.tile_pool(name="work", bufs=1))
        hbf_pool = ctx.enter_context(tc.tile_pool(name="hbf", bufs=2))
        psum_hb = ctx.enter_context(tc.tile_pool(name="psum_hb", bufs=1, space="PSUM"))
        psum_sm = ctx.enter_context(tc.tile_pool(name="psum_sm", bufs=1, space="PSUM"))
        psum_tp = ctx.enter_context(tc.tile_pool(name="psum_tp", bufs=1, space="PSUM"))

        # ---- persistent tiles --------------------------------------------
        W_sb = const_pool.tile([128, 12 * H4], bf16)
        nc.sync.dma_start(W_sb[:], Wc[:])
        b_sb = const_pool.tile([1, H4], bf16)
        nc.sync.dma_start(b_sb[:], bvec[:])
        id_sb = const_pool.tile([n, n], bf16)
        nc.sync.dma_start(id_sb[:], idb[:])
        A_ps = const_pool.tile([128, 4 * n * P16], bf16)   # [q, hc, p, s]
        nc.sync.dma_start(A_ps[:], ATps[:])
        A_sp = const_pool.tile([128, 4 * n * P16], bf16)   # [q, hc, s, p]
        nc.sync.dma_start(A_sp[:], ATsp[:])
        ones_r = const_pool.tile([1, 128], bf16)
        nc.sync.dma_start(ones_r[:], onesv[:])
        ones_k = const_pool.tile([128, 1], bf16)
        nc.sync.dma_start(ones_k[:], onesK[:])

        hT = const_pool.tile([128, 4 * n], bf16)           # [q, hc, s]
        nc.sync.dma_start(hT[:], h0T_in[:])
        c_st = const_pool.tile([n, H], bf16)
        nc.sync.dma_start(c_st[:], c0_in[:])

        Aps_v = A_ps[:].rearrange("q (c p s) -> q c p s", c=4, p=P16)
        Asp_v = A_sp[:].rearrange("q (c s p) -> q c s p", c=4, p=P16)
        hT_v = hT[:].rearrange("q (c s) -> q c s", c=4)
        xT_r = xT.rearrange("(c q) (nn tt) -> tt c q nn", q=128, tt=T)

        # scratch tiles reused every step
        prod = work.tile([128, 4 * n * P16], bf16)         # (p, s) order
        prod_v = prod[:].rearrange("q (c p s) -> q c p s", c=4, p=P16)
        wexp = work.tile([1, n * P16], bf16)               # (s, p) order
        w_bf = work.tile([128, n * P16], bf16)             # replicated wexp
        sc1 = work.tile([1, 8 * n], bf16)                  # ssum tree scratch
        ssum = work.tile([1, n], bf16)
        rinv = work.tile([1, n], bf16)
        rinv_rep = work.tile([128, n], bf16)
        prodA = work.tile([128, 4 * n * P16], bf16)        # (s, p) order
        prodA_v = prodA[:].rearrange("q (c s p) -> q c s p", c=4, p=P16)
        attnT_bf = work.tile([128, 4 * n], bf16)
        tau = work.tile([n, 3 * H], bf16)
        uu_o = work.tile([n, H], bf16)                     # 0.5*tau_o + 0.5
        gg = work.tile([n, H], bf16)
        m1 = work.tile([n, H], bf16)
        m2 = work.tile([n, H], bf16)
        tc_t = work.tile([n, H], bf16)
        acc_d = work.tile([n, 1], f32)
        u_i = work.tile([n, H], bf16)
        u_f = work.tile([n, H], bf16)

        W_k = lambda c, j: W_sb[:, H4 * c + 512 * j:H4 * c + 512 * (j + 1)]

        def emit_xb_mms(hb, xts, js):
            # x@Wx + b: first writes of the accumulation group (start=True
            # on the initial write of each column block)
            for j in js:
                cols = slice(512 * j, 512 * (j + 1))
                for ci, c in enumerate([8, 9, 10, 11, 12]):
                    if c < 12:
                        lhsT = xts[:, n * (c - 8):n * (c - 7)]
                        rhs = W_k(c, j)
                    else:
                        lhsT = ones_r[:, 0:n]
                        rhs = b_sb[:, cols]
                    nc.tensor.matmul(hb[:, cols], lhsT, rhs,
                                     start=(ci == 0), stop=False)

        # prologue: x/b matmuls for t=0
        xts_cur = xts_pool.tile([128, 4 * n], bf16)
        for ci in range(4):
            nc.sync.dma_start(xts_cur[:, n * ci:n * (ci + 1)], xT_r[0, ci])
        hb = psum_hb.tile([n, H4], f32)
        emit_xb_mms(hb, xts_cur, (0, 1, 2, 3))

        tp_prev = None
        for t in range(T):
            # prefetch x_{t+1}^T
            if t + 1 < T:
                xts_nxt = xts_pool.tile([128, 4 * n], bf16)
                for ci in range(4):
                    nc.sync.dma_start(xts_nxt[:, n * ci:n * (ci + 1)],
                                      xT_r[t + 1, ci])

            # ---- dot[p,s] = sum_h A[h,p,s] hT[h,s]; PE ones-reduce --------
            if NOBCAST:
                wrep2 = psum_sm.tile([128, n * P16], f32)
                dotp = wrep2[0:1, :]
            else:
                dotp = psum_sm.tile([1, n * P16], f32)
            half = n * P16 // 2
            # read h^T straight from the transpose PSUM (skips waiting on
            # the SBUF copy); first step reads the DMA'd h0T instead
            hsrc = hT_v if tp_prev is None else \
                tp_prev[:].rearrange("q (c s) -> q c s", c=4)
            if not STUBATTN:
                for cp in range(2):
                    h_b = hsrc[:, 2 * cp:2 * cp + 2] \
                        .rearrange("q c (r s) -> q c r s", r=1) \
                        .broadcast_to([128, 2, P16, n])
                    nc.vector.tensor_tensor(
                        prod_v[:, 2 * cp:2 * cp + 2],
                        Aps_v[:, 2 * cp:2 * cp + 2], h_b, ALU.mult)
                    for ci in (2 * cp, 2 * cp + 1):
                        for hi in range(2):
                            nc.tensor.matmul(
                                dotp[:, half * hi:half * (hi + 1)],
                                ones_k[:],
                                prod[:, n * P16 * ci + half * hi:
                                        n * P16 * ci + half * (hi + 1)],
                                start=(ci == 0),
                                stop=(ci == 3),
                            )

            # second half of next-psum x/b matmuls (PE filler behind the
            # ones-reduce; their psum group was opened last iteration)
            if t > 0:
                emit_xb_mms(hb, xts_cur, (2, 3))

            # h@Wh: fills PE while the softmax chain runs
            for j in range(4):
                cols = slice(512 * j, 512 * (j + 1))
                for c in range(4):
                    nc.tensor.matmul(hb[:, cols], hT[:, n * c:n * (c + 1)],
                                     W_k(c, j), start=False, stop=False)

            # ---- softmax numerator, stored (s, p) via strided write -------
            # two s-halves so the partition broadcast pipelines behind exp
            dview = dotp.rearrange("q (p s) -> q s p", p=P16)
            if STUBATTN:
                pass
            elif NOBCAST:
                nc.scalar.activation(
                    wexp[:].rearrange("q (s p) -> q s p", p=P16),
                    dview, ACTF.Exp, scale=SCALE)
                for hi in range(2):
                    sl = slice(half * hi, half * (hi + 1))
                    nc.tensor.matmul(wrep2[:, sl], ones_r[:], wexp[:, sl],
                                     start=True, stop=True)
                    nc.scalar.activation(w_bf[:, sl], wrep2[:, sl], ACTF.Copy)
            else:
                for hi in range(2):
                    sl = slice(half * hi, half * (hi + 1))
                    ssl = slice(32 * hi, 32 * (hi + 1))
                    nc.scalar.activation(
                        wexp[:, sl].rearrange("q (s p) -> q s p", p=P16),
                        dview[:, ssl],
                        ACTF.Exp, scale=SCALE)
                    nc.gpsimd.partition_broadcast(w_bf[:, sl], wexp[:, sl])

            # denominator: single-lane tree on wexp, then replicate rinv
            we_v = wexp[:].rearrange("q (s p) -> q s p", p=P16)
            sc_v = sc1[:].rearrange("q (s p) -> q s p", p=8)
            if not STUBATTN:
                with nc.allow_low_precision(reason="softmax sums in bf16"):
                    nc.vector.tensor_tensor(sc_v[:, :, 0:8], we_v[:, :, 0:8],
                                            we_v[:, :, 8:16], ALU.add)
                    nc.vector.tensor_tensor(sc_v[:, :, 0:4], sc_v[:, :, 0:4],
                                            sc_v[:, :, 4:8], ALU.add)
                    nc.vector.tensor_tensor(sc_v[:, :, 0:2], sc_v[:, :, 0:2],
                                            sc_v[:, :, 2:4], ALU.add)
                    nc.vector.tensor_tensor(
                        ssum[:].rearrange("q (s r) -> q s r", r=1),
                        sc_v[:, :, 0:1], sc_v[:, :, 1:2], ALU.add)
                    nc.vector.reciprocal(rinv[:], ssum[:])
            if STUBATTN:
                pass
            elif NOBCAST:
                rrep = psum_tp.tile([128, n], f32)
                nc.tensor.matmul(rrep[:], ones_r[:], rinv[:],
                                 start=True, stop=True)
                nc.scalar.activation(rinv_rep[:], rrep[:], ACTF.Copy)
            else:
                nc.gpsimd.partition_broadcast(rinv_rep[:], rinv[:])

            # ---- attn chunks: TT + in-place tree reduce + normalize -------
            # followed immediately by that chunk's 4 matmuls (c-outer);
            # chunks 1 and 3 multiply on Pool so DVE isn't the serial
            # bottleneck of the chunk phase
            w_v = w_bf[:].rearrange("q (s p) -> q s p", p=P16)
            with nc.allow_low_precision(reason="16-term attn sum in bf16"):
                if STUBATTN:
                    nc.scalar.activation(attnT_bf[:], hT[:], ACTF.Copy)
                for ci in range(4):
                    if STUBATTN:
                        pass
                    else:
                        pv = prodA_v[:, ci]
                        nc.vector.tensor_tensor(pv, Asp_v[:, ci], w_v,
                                                ALU.mult)
                        nc.vector.tensor_tensor(pv[:, :, 0:8], pv[:, :, 0:8],
                                                pv[:, :, 8:16], ALU.add)
                        nc.vector.tensor_tensor(pv[:, :, 0:4], pv[:, :, 0:4],
                                                pv[:, :, 4:8], ALU.add)
                        nc.vector.tensor_tensor(pv[:, :, 0:2], pv[:, :, 0:2],
                                                pv[:, :, 2:4], ALU.add)
                        nc.vector.tensor_tensor(pv[:, :, 0:1], pv[:, :, 0:1],
                                                pv[:, :, 1:2], ALU.add)
                        nc.vector.tensor_tensor(
                            attnT_bf[:, n * ci:n * (ci + 1)],
                            pv[:, :, 0],
                            rinv_rep[:], ALU.mult)
                    # this chunk's contribution to all column blocks; the
                    # last chunk's four matmuls run back-to-back BEFORE any
                    # gate reads hb (a gate read between them would force a
                    # tile-level WAR stall on the psum tile)
                    for j in (0, 1, 3, 2):
                        cols = slice(512 * j, 512 * (j + 1))
                        nc.tensor.matmul(
                            hb[:, cols],
                            attnT_bf[:, n * ci:n * (ci + 1)],
                            W_k(4 + ci, j),
                            start=False, stop=(ci == 3))

            # ---- gates; sigma(z) = 0.5*tanh(z/2) + 0.5 --------------------
            nc.scalar.activation(tau[:, 0:2 * H], hb[:, 0:2 * H],
                                 ACTF.Tanh, scale=0.5)
            nc.scalar.activation(gg[:], hb[:, 3 * H:4 * H], ACTF.Tanh)
            nc.scalar.activation(tau[:, 2 * H:3 * H], hb[:, 2 * H:3 * H],
                                 ACTF.Tanh, scale=0.5)
            nc.vector.tensor_scalar(uu_o[:], tau[:, 2 * H:3 * H],
                                    0.5, 0.5, ALU.mult, ALU.add)

            # ---- cell: c' = sig(f)c + sig(i)g ; h = sig(o)*tanh(c') -------
            # (plain TS/TT ops only: scalar_tensor_tensor and
            # tensor_tensor_reduce fault on hardware here)
            with nc.allow_low_precision(reason="cell state in bf16"):
                nc.vector.tensor_scalar(u_i[:], tau[:, 0:H],
                                        0.5, 0.5, ALU.mult, ALU.add)
                nc.vector.tensor_scalar(u_f[:], tau[:, H:2 * H],
                                        0.5, 0.5, ALU.mult, ALU.add)
                nc.vector.tensor_tensor(m2[:], u_i[:], gg[:], ALU.mult)
                nc.vector.tensor_tensor(m1[:], u_f[:], c_st[:], ALU.mult)
                nc.vector.tensor_tensor(c_st[:], m1[:], m2[:], ALU.add)
            nc.scalar.activation(tc_t[:], c_st[:], ACTF.Tanh)
            h_bf = hbf_pool.tile([n, H], bf16)
            nc.vector.tensor_tensor(h_bf[:], uu_o[:], tc_t[:], ALU.mult)

            nc.sync.dma_start(hs[t], h_bf[:])

            # transpose h for next step, then prime next step's x/b matmuls
            if t < T - 1:
                tp = psum_tp.tile([128, 4 * n], bf16)
                for ci in range(4):
                    nc.tensor.transpose(
                        tp[:, n * ci:n * (ci + 1)],
                        h_bf[:, 128 * ci:128 * (ci + 1)], id_sb[:])
                nc.scalar.activation(hT[:], tp[:], ACTF.Copy)
                tp_prev = tp
                hb = psum_hb.tile([n, H4], f32)
                emit_xb_mms(hb, xts_nxt, (0, 1))
                xts_cur = xts_nxt

    nc.compile()
    _cache["nc"] = nc
    return nc


def kernel(x, A, Wx, Wh, Wattn, b):
    import ml_dtypes
    from concourse import bass_utils

    nc = _build_kernel()
    bft = ml_dtypes.bfloat16

    Wcat = np.concatenate([np.asarray(Wh), np.asarray(Wattn), np.asarray(Wx)],
                          axis=0)                         # (1536, 2048)
    Wc_host = np.ascontiguousarray(
        Wcat.reshape(12, 128, H4).transpose(1, 0, 2).reshape(128, 12 * H4)
    ).astype(bft)
    b_host = np.asarray(b, dtype=np.float32).reshape(1, H4).astype(bft)
    id_host = np.eye(n, dtype=np.float32).astype(bft)
    ones_host = np.ones((1, 128), dtype=np.float32).astype(bft)
    onesK_host = np.ones((128, 1), dtype=np.float32).astype(bft)

    A_np = np.asarray(A, dtype=np.float32)                # (N, H, 4, 4)
    x_np = np.asarray(x, dtype=np.float32)[:, :T]

    in_maps = []
    for k in range(NC):
        xc = x_np[n * k:n * (k + 1)]                      # (64, T, D)
        Ac = A_np[n * k:n * (k + 1)].reshape(n, H, P16)   # (64, 512, 16)
        xT_host = np.ascontiguousarray(
            xc.transpose(2, 0, 1).reshape(D, n * T)).astype(bft)
        A_hps = Ac.transpose(1, 2, 0)                     # (H, p, s)
        ATps_host = np.ascontiguousarray(
            A_hps.reshape(4, 128, P16 * n)
                 .transpose(1, 0, 2).reshape(128, 4 * n * P16)).astype(bft)
        A_hsp = Ac.transpose(1, 0, 2)                     # (H, s, p)
        ATsp_host = np.ascontiguousarray(
            A_hsp.reshape(4, 128, n * P16)
                 .transpose(1, 0, 2).reshape(128, 4 * n * P16)).astype(bft)
        h0 = Ac.mean(axis=2)                              # (64, 512)
        h0T_host = np.ascontiguousarray(
            h0.T.reshape(4, 128, n).transpose(1, 0, 2).reshape(128, 4 * n)
        ).astype(bft)
        in_maps.append({
            "xT": xT_host,
            "ATps": ATps_host,
            "ATsp": ATsp_host,
            "Wc": Wc_host,
            "bvec": b_host,
            "idb": id_host,
            "onesv": ones_host,
            "onesK": onesK_host,
            "h0T": h0T_host,
            "c0": np.ascontiguousarray(h0).astype(bft),
        })

    global LAST_EXEC_NS
    res = bass_utils.run_bass_kernel_spmd(
        nc, in_maps, core_ids=list(range(NC)), trace=TRACE)
    LAST_EXEC_NS = res.exec_time_ns

    out = np.empty((N, T, H), dtype=np.float32)
    for k in range(NC):
        hs_k = np.asarray(res.results[k]["hs"]).astype(np.float32)  # (T, n, H)
        out[n * k:n * (k + 1)] = hs_k.transpose(1, 0, 2)
    return out


# revision 60
# speedup vs baseline: 1.0348x; 1.0241x over previous
import math
import os
from contextlib import ExitStack

import numpy as np

N, T, D, H = 512, 128, 512, 512
NC = 8
n = N // NC          # 64 samples per core
H4 = 4 * H           # 2048
P16 = 16             # attention locations
SCALE = 1.0 / math.sqrt(H)
TRACE = os.environ.get("KERNEL_TRACE", "0") == "1"
NOBCAST = False
STUBATTN = False
SIMPLECELL = True
LAST_EXEC_NS = None

_cache = {}


def _build_kernel():
    if "nc" in _cache:
        return _cache["nc"]

    import concourse.bass as bass
    import concourse.tile as tile
    from concourse import bacc, mybir

    f32 = mybir.dt.float32
    bf16 = mybir.dt.bfloat16
    ALU = mybir.AluOpType
    ACTF = mybir.ActivationFunctionType
    AX = mybir.AxisListType

    nc = bacc.Bacc(
        "TRN2",
        target_bir_lowering=False,
        debug=False,
        enable_asserts=False,
        num_devices=NC,
    )

    # ---- DRAM I/O ---------------------------------------------------------
    xT = nc.dram_tensor("xT", (D, n * T), bf16, kind="ExternalInput").ap()
    # A^T chunks in (p, s) free order for the dot product, and (s, p)
    # order for the weighted-sum path (keeps innermost stride 1 on both).
    ATps = nc.dram_tensor("ATps", (128, 4 * n * P16), bf16, kind="ExternalInput").ap()
    ATsp = nc.dram_tensor("ATsp", (128, 4 * n * P16), bf16, kind="ExternalInput").ap()
    Wc = nc.dram_tensor("Wc", (128, 12 * H4), bf16, kind="ExternalInput").ap()
    bvec = nc.dram_tensor("bvec", (1, H4), bf16, kind="ExternalInput").ap()
    idb = nc.dram_tensor("idb", (n, n), bf16, kind="ExternalInput").ap()
    onesv = nc.dram_tensor("onesv", (1, 128), bf16, kind="ExternalInput").ap()
    onesK = nc.dram_tensor("onesK", (128, 1), bf16, kind="ExternalInput").ap()
    onesKK = nc.dram_tensor("onesKK", (128, 128), bf16, kind="ExternalInput").ap()
    h0T_in = nc.dram_tensor("h0T", (128, 4 * n), bf16, kind="ExternalInput").ap()
    c0_in = nc.dram_tensor("c0", (n, H), bf16, kind="ExternalInput").ap()
    hs = nc.dram_tensor("hs", (T, n, H), bf16, kind="ExternalOutput").ap()

    with tile.TileContext(nc) as tc, ExitStack() as ctx:
        const_pool = ctx.enter_context(tc.tile_pool(name="const", bufs=1))
        xts_pool = ctx.enter_context(tc.tile_pool(name="xts", bufs=3))
        work = ctx.enter_context(tc.tile_pool(name="work", bufs=1))
        hbf_pool = ctx.enter_context(tc.tile_pool(name="hbf", bufs=2))
        psum_hb = ctx.enter_context(tc.tile_pool(name="psum_hb", bufs=1, space="PSUM"))
        psum_sm = ctx.enter_context(tc.tile_pool(name="psum_sm", bufs=1, space="PSUM"))
        psum_tp = ctx.enter_context(tc.tile_pool(name="psum_tp", bufs=1, space="PSUM"))

        # ---- persistent tiles --------------------------------------------
        W_sb = const_pool.tile([128, 12 * H4], bf16)
        nc.sync.dma_start(W_sb[:], Wc[:])
        b_sb = const_pool.tile([1, H4], bf16)
        nc.sync.dma_start(b_sb[:], bvec[:])
        id_sb = const_pool.tile([n, n], bf16)
        nc.sync.dma_start(id_sb[:], idb[:])
        A_ps = const_pool.tile([128, 4 * n * P16], bf16)   # [q, hc, p, s]
        nc.sync.dma_start(A_ps[:], ATps[:])
        A_sp = const_pool.tile([128, 4 * n * P16], bf16)   # [q, hc, s, p]
        nc.sync.dma_start(A_sp[:], ATsp[:])
        ones_r = const_pool.tile([1, 128], bf16)
        nc.sync.dma_start(ones_r[:], onesv[:])
        ones_k = const_pool.tile([128, 1], bf16)
        nc.sync.dma_start(ones_k[:], onesK[:])
        ones_kk = const_pool.tile([128, 128], bf16)
        nc.sync.dma_start(ones_kk[:], onesKK[:])

        hT = const_pool.tile([128, 4 * n], bf16)           # [q, hc, s]
        nc.sync.dma_start(hT[:], h0T_in[:])
        c_st = const_pool.tile([n, H], bf16)
        nc.sync.dma_start(c_st[:], c0_in[:])

        Aps_v = A_ps[:].rearrange("q (c p s) -> q c p s", c=4, p=P16)
        Asp_v = A_sp[:].rearrange("q (c s p) -> q c s p", c=4, p=P16)
        hT_v = hT[:].rearrange("q (c s) -> q c s", c=4)
        xT_r = xT.rearrange("(c q) (nn tt) -> tt c q nn", q=128, tt=T)

        # scratch tiles reused every step
        prod = work.tile([128, 4 * n * P16], bf16)         # (p, s) order
        prod_v = prod[:].rearrange("q (c p s) -> q c p s", c=4, p=P16)
        w_bf = work.tile([128, n * P16], bf16)             # softmax weights
        sc1 = work.tile([128, 8 * n], bf16)                # ssum tree scratch
        rinv_rep = work.tile([128, n], bf16)
        prodA = work.tile([128, 4 * n * P16], bf16)        # (s, p) order
        prodA_v = prodA[:].rearrange("q (c s p) -> q c s p", c=4, p=P16)
        attnT_bf = work.tile([128, 4 * n], bf16)
        tau = work.tile([n, 3 * H], bf16)
        uu_o = work.tile([n, H], bf16)                     # 0.5*tau_o + 0.5
        gg = work.tile([n, H], bf16)
        m1 = work.tile([n, H], bf16)
        m2 = work.tile([n, H], bf16)
        tc_t = work.tile([n, H], bf16)
        acc_d = work.tile([n, 1], f32)
        u_i = work.tile([n, H], bf16)
        u_f = work.tile([n, H], bf16)

        W_k = lambda c, j: W_sb[:, H4 * c + 512 * j:H4 * c + 512 * (j + 1)]

        def emit_xb_mms(hb, xts, js):
            # x@Wx + b: first writes of the accumulation group (start=True
            # on the initial write of each column block)
            for j in js:
                cols = slice(512 * j, 512 * (j + 1))
                for ci, c in enumerate([8, 9, 10, 11, 12]):
                    if c < 12:
                        lhsT = xts[:, n * (c - 8):n * (c - 7)]
                        rhs = W_k(c, j)
                    else:
                        lhsT = ones_r[:, 0:n]
                        rhs = b_sb[:, cols]
                    nc.tensor.matmul(hb[:, cols], lhsT, rhs,
                                     start=(ci == 0), stop=False)

        # prologue: x/b matmuls for t=0
        xts_cur = xts_pool.tile([128, 4 * n], bf16)
        for ci in range(4):
            nc.sync.dma_start(xts_cur[:, n * ci:n * (ci + 1)], xT_r[0, ci])
        hb = psum_hb.tile([n, H4], f32)
        emit_xb_mms(hb, xts_cur, (0, 1, 2, 3))

        tp_prev = None
        for t in range(T):
            # prefetch x_{t+1}^T
            if t + 1 < T:
                xts_nxt = xts_pool.tile([128, 4 * n], bf16)
                for ci in range(4):
                    nc.sync.dma_start(xts_nxt[:, n * ci:n * (ci + 1)],
                                      xT_r[t + 1, ci])

            # ---- dot[p,s] = sum_h A[h,p,s] hT[h,s]; PE ones-reduce --------
            dot_rep = psum_sm.tile([128, n * P16], f32)
            half = n * P16 // 2
            # read h^T straight from the transpose PSUM (skips waiting on
            # the SBUF copy); first step reads the DMA'd h0T instead
            hsrc = hT_v if tp_prev is None else \
                tp_prev[:].rearrange("q (c s) -> q c s", c=4)
            if not STUBATTN:
                for cp in range(2):
                    h_b = hsrc[:, 2 * cp:2 * cp + 2] \
                        .rearrange("q c (r s) -> q c r s", r=1) \
                        .broadcast_to([128, 2, P16, n])
                    nc.vector.tensor_tensor(
                        prod_v[:, 2 * cp:2 * cp + 2],
                        Aps_v[:, 2 * cp:2 * cp + 2], h_b, ALU.mult)
                    for ci in (2 * cp, 2 * cp + 1):
                        for hi in range(2):
                            nc.tensor.matmul(
                                dot_rep[:, half * hi:half * (hi + 1)],
                                ones_kk[:],
                                prod[:, n * P16 * ci + half * hi:
                                        n * P16 * ci + half * (hi + 1)],
                                start=(ci == 0),
                                stop=(ci == 3),
                            )

            # second half of next-psum x/b matmuls (PE filler behind the
            # ones-reduce; their psum group was opened last iteration)
            if t > 0:
                emit_xb_mms(hb, xts_cur, (2, 3))

            # h@Wh: fills PE while the softmax chain runs
            for j in range(4):
                cols = slice(512 * j, 512 * (j + 1))
                for c in range(4):
                    nc.tensor.matmul(hb[:, cols], hT[:, n * c:n * (c + 1)],
                                     W_k(c, j), start=False, stop=False)

            # ---- softmax numerator: exp of the (already replicated)
            # dot rows, written in (s, p) order via strided read ------------
            dview = dot_rep.rearrange("q (p s) -> q s p", p=P16)
            for hi in range(2):
                sl = slice(half * hi, half * (hi + 1))
                ssl = slice(32 * hi, 32 * (hi + 1))
                nc.scalar.activation(
                    w_bf[:, sl].rearrange("q (s p) -> q s p", p=P16),
                    dview[:, ssl],
                    ACTF.Exp, scale=SCALE)

            # denominator: full-width tree on the replicated weights
            we_v = w_bf[:].rearrange("q (s p) -> q s p", p=P16)
            sc_v = sc1[:].rearrange("q (s p) -> q s p", p=8)
            with nc.allow_low_precision(reason="softmax sums in bf16"):
                nc.vector.tensor_tensor(sc_v[:, :, 0:8], we_v[:, :, 0:8],
                                        we_v[:, :, 8:16], ALU.add)
                nc.vector.tensor_tensor(sc_v[:, :, 0:4], sc_v[:, :, 0:4],
                                        sc_v[:, :, 4:8], ALU.add)
                nc.vector.tensor_tensor(sc_v[:, :, 0:2], sc_v[:, :, 0:2],
                                        sc_v[:, :, 2:4], ALU.add)
                nc.vector.tensor_tensor(
                    rinv_rep[:].rearrange("q (s r) -> q s r", r=1),
                    sc_v[:, :, 0:1], sc_v[:, :, 1:2], ALU.add)
                nc.vector.reciprocal(rinv_rep[:], rinv_rep[:])

            # ---- attn chunks: TT + in-place tree reduce + normalize -------
            # followed immediately by that chunk's 4 matmuls (c-outer);
            # chunks 1 and 3 multiply on Pool so DVE isn't the serial
            # bottleneck of the chunk phase
            w_v = w_bf[:].rearrange("q (s p) -> q s p", p=P16)
            with nc.allow_low_precision(reason="16-term attn sum in bf16"):
                if STUBATTN:
                    nc.scalar.activation(attnT_bf[:], hT[:], ACTF.Copy)
                for ci in range(4):
                    if STUBATTN:
                        pass
                    else:
                        pv = prodA_v[:, ci]
                        nc.vector.tensor_tensor(pv, Asp_v[:, ci], w_v,
                                                ALU.mult)
                        nc.vector.tensor_tensor(pv[:, :, 0:8], pv[:, :, 0:8],
                                                pv[:, :, 8:16], ALU.add)
                        nc.vector.tensor_tensor(pv[:, :, 0:4], pv[:, :, 0:4],
                                                pv[:, :, 4:8], ALU.add)
                        nc.vector.tensor_tensor(pv[:, :, 0:2], pv[:, :, 0:2],
                                                pv[:, :, 2:4], ALU.add)
                        nc.vector.tensor_tensor(pv[:, :, 0:1], pv[:, :, 0:1],
                                                pv[:, :, 1:2], ALU.add)
                        nc.vector.tensor_tensor(
                            attnT_bf[:, n * ci:n * (ci + 1)],
                            pv[:, :, 0],
                            rinv_rep[:], ALU.mult)
                    # this chunk's contribution to all column blocks; the
                    # last chunk's four matmuls run back-to-back BEFORE any
                    # gate reads hb (a gate read between them would force a
                    # tile-level WAR stall on the psum tile)
                    for j in (0, 1, 3, 2):
                        cols = slice(512 * j, 512 * (j + 1))
                        nc.tensor.matmul(
                            hb[:, cols],
                            attnT_bf[:, n * ci:n * (ci + 1)],
                            W_k(4 + ci, j),
                            start=False, stop=(ci == 3))

            # ---- gates; sigma(z) = 0.5*tanh(z/2) + 0.5 --------------------
            nc.scalar.activation(tau[:, 0:2 * H], hb[:, 0:2 * H],
                                 ACTF.Tanh, scale=0.5)
            nc.scalar.activation(gg[:], hb[:, 3 * H:4 * H], ACTF.Tanh)
            nc.scalar.activation(tau[:, 2 * H:3 * H], hb[:, 2 * H:3 * H],
                                 ACTF.Tanh, scale=0.5)
            nc.vector.tensor_scalar(uu_o[:], tau[:, 2 * H:3 * H],
                                    0.5, 0.5, ALU.mult, ALU.add)

            # ---- cell: c' = sig(f)c + sig(i)g ; h = sig(o)*tanh(c') -------
            # (plain TS/TT ops only: scalar_tensor_tensor and
            # tensor_tensor_reduce fault on hardware here)
            with nc.allow_low_precision(reason="cell state in bf16"):
                nc.vector.tensor_scalar(u_i[:], tau[:, 0:H],
                                        0.5, 0.5, ALU.mult, ALU.add)
                nc.vector.tensor_scalar(u_f[:], tau[:, H:2 * H],
                                        0.5, 0.5, ALU.mult, ALU.add)
                nc.vector.tensor_tensor(m2[:], u_i[:], gg[:], ALU.mult)
                nc.vector.tensor_tensor(m1[:], u_f[:], c_st[:], ALU.mult)
                nc.vector.tensor_tensor(c_st[:], m1[:], m2[:], ALU.add)
            nc.scalar.activation(tc_t[:], c_st[:], ACTF.Tanh)
            h_bf = hbf_pool.tile([n, H], bf16)
            nc.vector.tensor_tensor(h_bf[:], uu_o[:], tc_t[:], ALU.mult)

            nc.sync.dma_start(hs[t], h_bf[:])

            # transpose h for next step, then prime next step's x/b matmuls
            if t < T - 1:
                tp = psum_tp.tile([128, 4 * n], bf16)
                for ci in range(4):
                    nc.tensor.transpose(
                        tp[:, n * ci:n * (ci + 1)],
                        h_bf[:, 128 * ci:128 * (ci + 1)], id_sb[:])
                nc.scalar.activation(hT[:], tp[:], ACTF.Copy)
                tp_prev = tp
                hb = psum_hb.tile([n, H4], f32)
                emit_xb_mms(hb, xts_nxt, (0, 1))
                xts_cur = xts_nxt

    nc.compile()
    _cache["nc"] = nc
    return nc


def kernel(x, A, Wx, Wh, Wattn, b):
    import ml_dtypes
    from concourse import bass_utils

    nc = _build_kernel()
    bft = ml_dtypes.bfloat16

    Wcat = np.concatenate([np.asarray(Wh), np.asarray(Wattn), np.asarray(Wx)],
                          axis=0)                         # (1536, 2048)
    Wc_host = np.ascontiguousarray(
        Wcat.reshape(12, 128, H4).transpose(1, 0, 2).reshape(128, 12 * H4)
    ).astype(bft)
    b_host = np.asarray(b, dtype=np.float32).reshape(1, H4).astype(bft)
    id_host = np.eye(n, dtype=np.float32).astype(bft)
    ones_host = np.ones((1, 128), dtype=np.float32).astype(bft)
    onesK_host = np.ones((128, 1), dtype=np.float32).astype(bft)
    onesKK_host = np.ones((128, 128), dtype=np.float32).astype(bft)

    A_np = np.asarray(A, dtype=np.float32)                # (N, H, 4, 4)
    x_np = np.asarray(x, dtype=np.float32)[:, :T]

    in_maps = []
    for k in range(NC):
        xc = x_np[n * k:n * (k + 1)]                      # (64, T, D)
        Ac = A_np[n * k:n * (k + 1)].reshape(n, H, P16)   # (64, 512, 16)
        xT_host = np.ascontiguousarray(
            xc.transpose(2, 0, 1).reshape(D, n * T)).astype(bft)
        A_hps = Ac.transpose(1, 2, 0)                     # (H, p, s)
        ATps_host = np.ascontiguousarray(
            A_hps.reshape(4, 128, P16 * n)
                 .transpose(1, 0, 2).reshape(128, 4 * n * P16)).astype(bft)
        A_hsp = Ac.transpose(1, 0, 2)                     # (H, s, p)
        ATsp_host = np.ascontiguousarray(
            A_hsp.reshape(4, 128, n * P16)
                 .transpose(1, 0, 2).reshape(128, 4 * n * P16)).astype(bft)
        h0 = Ac.mean(axis=2)                              # (64, 512)
        h0T_host = np.ascontiguousarray(
            h0.T.reshape(4, 128, n).transpose(1, 0, 2).reshape(128, 4 * n)
        ).astype(bft)
        in_maps.append({
            "xT": xT_host,
            "ATps": ATps_host,
            "ATsp": ATsp_host,
            "Wc": Wc_host,
            "bvec": b_host,
            "idb": id_host,
            "onesv": ones_host,
            "onesK": onesK_host,
            "onesKK": onesKK_host,
            "h0T": h0T_host,
            "c0": np.ascontiguousarray(h0).astype(bft),
        })

    global LAST_EXEC_NS
    res = bass_utils.run_bass_kernel_spmd(
        nc, in_maps, core_ids=list(range(NC)), trace=TRACE)
    LAST_EXEC_NS = res.exec_time_ns

    out = np.empty((N, T, H), dtype=np.float32)
    for k in range(NC):
        hs_k = np.asarray(res.results[k]["hs"]).astype(np.float32)  # (T, n, H)
        out[n * k:n * (k + 1)] = hs_k.transpose(1, 0, 2)
    return out
